# revision 2
# baseline (speedup 1.0000x reference)
"""Trainium2 Bass kernel for batched tanh-attention flat-softmax.

Per batch b:
    Q = query[b] @ W_query; K = query[b] @ W_key      # [S, 64]
    s = tanh(Q @ K.T) * 10                            # [S, S]
    s[diag] = -inf
    out[b] = softmax(s.flatten())

Sharding: data-parallel over batch across 8 NeuronCores (6 batches/core),
W_query/W_key replicated; no cross-core communication.

Numerics: tanh(x)*10 is bounded in [-10,10], so softmax needs no max
subtraction: out = exp(10*tanh(s)) / sum(...). The diagonal is clamped to
-1e9 on the raw scores (pre-tanh), so tanh -> -1 and exp -> e^-10, which
after the 1/Z normalize is ~5e-15 (reference has exactly 0; the difference
is far below the accuracy target).

Precision strategy (validated vs fp32 reference: rel L2 ~6e-3):
  - query is rounded once to bf16 (xh); its lo half is dropped.
  - W is kept as bf16 hi+lo ([Wq|Wk] stacked): proj = wh.T xhT + wl.T xhT
    accumulated in fp32 PSUM -> pp = [Q; K] with k on partitions.
  - scores = Qh.T Kh in pure bf16 (64-contraction), no correction terms.
  - xhT is produced by PE transposes (identity matmul) into a bf16 PSUM
    scratch -- no DRAM-roundtrip DMA transpose.

Engine budget per core (target): scalar ~88us (tanh+exp, the roofline),
DMA ~78us (24MB output store dominated), PE ~45us, DVE ~60us.
"""

import numpy as np

import concourse.bass as bass
import concourse.bass_isa as bass_isa
import concourse.mybir as mybir
import concourse.tile as tile
from concourse import bacc
from concourse.bass_utils import run_bass_kernel_spmd

B = 48
S = 1024
D = 128
DK = 64
N_CORES = 8
BPC = B // N_CORES
P = 128
NQ = S // P
F32 = mybir.dt.float32
BF16 = mybir.dt.bfloat16
AL = mybir.AluOpType

TANH_CLIP = 10.0
DIAG_NEG = -1.0e9  # pre-tanh diag clamp; tanh(-1e9) = -1 -> exp(-10) ~ 0

NSLOT = 3  # PSUM score ring depth (chunks of [P, S] fp32, 2 banks each)


def build_bass() -> bass.Bass:
    nc = bacc.Bacc(None, target_bir_lowering=False)

    q_d = nc.dram_tensor("query", [BPC, S, D], F32, kind="ExternalInput")
    wq_d = nc.dram_tensor("W_query", [D, DK], F32, kind="ExternalInput")
    wk_d = nc.dram_tensor("W_key", [D, DK], F32, kind="ExternalInput")
    out_d = nc.dram_tensor("out", [BPC, S, S], F32, kind="ExternalOutput")

    with tile.TileContext(nc) as tc:
        with (
            tc.tile_pool(name="singles", bufs=1) as singles,
            tc.tile_pool(name="qload", bufs=2) as qload,
            tc.tile_pool(name="xhp", bufs=2) as xhp,
            tc.tile_pool(name="xhtp", bufs=2) as xhtp,
            tc.tile_pool(name="hbp", bufs=2) as hbp,
            tc.tile_pool(name="tbuf", bufs=3) as tbuf,
            tc.tile_pool(name="small", bufs=2) as small,
            tc.tile_pool(name="psum1", bufs=1, space="PSUM") as psum1,
        ):
            # --- one-time setup ---
            # bf16 identity for PE transposes
            ident = singles.tile([P, P], BF16)
            nc.vector.memset(ident, 0.0)
            nc.gpsimd.affine_select(
                out=ident,
                in_=ident,
                compare_op=AL.not_equal,
                fill=1.0,
                base=0,
                pattern=[[-1, P]],
                channel_multiplier=1,
            )
            # diag clamp mask: min(scores, dmask) forces diagonal to -1e9
            dmask = singles.tile([P, P], F32)
            nc.vector.memset(dmask, 3.0e38)
            nc.gpsimd.affine_select(
                out=dmask,
                in_=dmask,
                compare_op=AL.not_equal,
                fill=DIAG_NEG,
                base=0,
                pattern=[[-1, P]],
                channel_multiplier=1,
            )

            # W stacked [Wq | Wk] as fp32, then bf16 hi/lo
            w32 = singles.tile([D, 2 * DK], F32)
            nc.sync.dma_start(w32[:, 0:DK], wq_d[:, :])
            nc.sync.dma_start(w32[:, DK:2 * DK], wk_d[:, :])
            wh = singles.tile([D, 2 * DK], BF16)
            nc.vector.tensor_copy(wh, w32)
            wl = singles.tile([D, 2 * DK], BF16)
            nc.vector.tensor_tensor(wl, w32, wh, AL.subtract)

            # --- persistent PSUM carve-up (exactly 8 banks) ---
            # score ring: 3 slots x [P, S] fp32 (2 banks each)
            sc_all = psum1.tile([P, NSLOT * S], F32)
            # PE-transpose scratch: [P, S] bf16 (1 bank)
            tp = psum1.tile([P, S], BF16)
            # projection accumulator: [P, 512] fp32 (1 bank), used twice/batch
            pp = psum1.tile([P, 512], F32)

            def load_q(b):
                q_sb = qload.tile([P, NQ, D], F32, tag="q")
                nc.sync.dma_start(
                    q_sb, q_d[b].rearrange("(n p) d -> p n d", p=P)
                )
                return q_sb

            def front_end_a(q_sb):
                """bf16-cast query, PE-transpose all 8 chunks -> xhT."""
                xh = xhp.tile([P, NQ, D], BF16, tag="xh")
                nc.vector.tensor_copy(xh, q_sb)
                for c in range(NQ):
                    nc.tensor.transpose(
                        tp[:, c * P:(c + 1) * P], xh[:, c, :], ident
                    )
                xhT = xhtp.tile([P, S], BF16, tag="xhT")
                nc.vector.tensor_copy(xhT, tp)
                return xhT

            def front_end_b(xhT):
                """proj pp = [Q;K] (fp32 psum) -> hb bf16; kh = dup of Kh."""
                hb = hbp.tile([P, S], BF16, tag="hb")
                kh = hbp.tile([P, S], BF16, tag="kh")
                for h in range(2):
                    cols = slice(h * 512, (h + 1) * 512)
                    nc.tensor.matmul(pp, wh, xhT[:, cols], start=True, stop=False)
                    nc.tensor.matmul(pp, wl, xhT[:, cols], start=False, stop=True)
                    nc.vector.tensor_copy(hb[:, cols], pp)
                    nc.vector.tensor_copy(kh[0:DK, cols], hb[DK:P, cols])
                return hb, kh

            def score_chunk(hb, kh, c):
                """scores chunk c -> PSUM slot c%NSLOT; diag-clamp if needed."""
                slot = c % NSLOT
                scc = sc_all[:, slot * S:(slot + 1) * S]
                lhsT = hb[0:DK, c * P:(c + 1) * P]
                for h in range(2):
                    cols = slice(h * 512, (h + 1) * 512)
                    nc.tensor.matmul(
                        scc[:, cols], lhsT, kh[0:DK, cols], start=True, stop=True
                    )
                # clamp this chunk's diagonal block (pre-tanh, off the
                # scalar critical path)
                nc.vector.tensor_tensor(
                    scc[:, c * P:(c + 1) * P],
                    scc[:, c * P:(c + 1) * P],
                    dmask,
                    AL.min,
                )

            def tanh_chunks(t_sb, c0, n):
                """tanh PSUM slots [c0 % NSLOT ... +n) -> t_sb chunks c0..c0+n.
                Caller guarantees the slots are ascending-contiguous."""
                slot = c0 % NSLOT
                nc.scalar.activation(
                    out=t_sb[:, c0:c0 + n],
                    in_=sc_all[:, slot * S:(slot + n) * S],
                    func=mybir.ActivationFunctionType.Tanh,
                )

            # tanh batching for chunks 0..7 on the 3-slot ring:
            # chunks (0,1)->slots(0,1) paired; 2->slot2, 3->slot0 singles;
            # (4,5)->slots(1,2) paired; (6,7)->slots(0,1) paired.
            TANH_GROUPS = [(0, 2), (2, 1), (3, 1), (4, 2), (6, 2)]

            def scores_and_tanh(hb, kh, t_sb, groups):
                for c0, n in groups:
                    for c in range(c0, c0 + n):
                        score_chunk(hb, kh, c)
                    tanh_chunks(t_sb, c0, n)

            def exp_batch(t_sb):
                """exp(10*t) in place over the whole batch, row sums -> rs."""
                rs = small.tile([P, 1], F32, tag="rs")
                nc.scalar.activation(
                    out=t_sb,
                    in_=t_sb,
                    func=mybir.ActivationFunctionType.Exp,
                    scale=TANH_CLIP,
                    accum_out=rs,
                )
                zall = small.tile([P, 1], F32, tag="zall")
                nc.gpsimd.partition_all_reduce(
                    zall, rs, channels=P, reduce_op=bass_isa.ReduceOp.add
                )
                rz = small.tile([P, 1], F32, tag="rz")
                nc.vector.reciprocal(rz, zall)
                return rz

            def normalize_store(b, t_sb, rz):
                nc.vector.tensor_scalar_mul(t_sb, t_sb, rz)
                # big store via SWDGE on the (otherwise idle) GpSimd queue
                nc.gpsimd.dma_start(
                    out_d[b].rearrange("(n p) s -> p n s", p=P), t_sb
                )

            # ---- software-pipelined batch loop --------------------------
            # prologue: batch 0 front-end
            q_sb = load_q(0)
            xhT = front_end_a(q_sb)
            ops = front_end_b(xhT)
            pending = None  # (b, t_sb, rz) awaiting normalize+store

            for b in range(BPC):
                t_sb = tbuf.tile([P, NQ, S], F32, tag="t")
                hb, kh = ops

                if b + 1 < BPC:
                    q_next = load_q(b + 1)

                # first half of this batch's scores/tanh
                scores_and_tanh(hb, kh, t_sb, TANH_GROUPS[:3])

                # previous batch normalize+store: DVE executes it while the
                # scalar engine streams this batch's tanh
                if pending is not None:
                    normalize_store(*pending)
                    pending = None

                # next batch front-end part 1 (cast + PE transposes)
                if b + 1 < BPC:
                    xhT_next = front_end_a(q_next)

                # second half of scores/tanh
                scores_and_tanh(hb, kh, t_sb, TANH_GROUPS[3:])

                # next batch front-end part 2 (proj + hb/kh)
                if b + 1 < BPC:
                    ops = front_end_b(xhT_next)

                rz = exp_batch(t_sb)
                pending = (b, t_sb, rz)

            normalize_store(*pending)

    nc.compile()
    return nc


_CACHED_NC = None


def kernel(**inputs: np.ndarray) -> np.ndarray:
    global _CACHED_NC
    query = np.ascontiguousarray(np.asarray(inputs["query"], dtype=np.float32))
    wq = np.ascontiguousarray(np.asarray(inputs["W_query"], dtype=np.float32))
    wk = np.ascontiguousarray(np.asarray(inputs["W_key"], dtype=np.float32))
    assert query.shape == (B, S, D), query.shape

    if _CACHED_NC is None:
        _CACHED_NC = build_bass()
    nc = _CACHED_NC

    in_maps = [
        {
            "query": query[c * BPC:(c + 1) * BPC],
            "W_query": wq,
            "W_key": wk,
        }
        for c in range(N_CORES)
    ]
    res = run_bass_kernel_spmd(nc, in_maps, core_ids=list(range(N_CORES)))
    out = np.concatenate(
        [r["out"].reshape(BPC, S * S) for r in res.results], axis=0
    )
    return out


# revision 3
# speedup vs baseline: 1.3802x; 1.3802x over previous
"""Trainium2 Bass kernel for batched tanh-attention flat-softmax.

Per batch b:
    Q = query[b] @ W_query; K = query[b] @ W_key      # [S, 64]
    s = tanh(Q @ K.T) * 10                            # [S, S]
    s[diag] = -inf
    out[b] = softmax(s.flatten())

Sharding: data-parallel over batch across 8 NeuronCores (6 batches/core),
W_query/W_key replicated; no cross-core communication.

Numerics: tanh(x)*10 is bounded in [-10,10], so softmax needs no max
subtraction: out = exp(10*tanh(s)) / sum(...). The diagonal is clamped to
-1e9 on the raw scores (pre-tanh, off the scalar-engine critical path), so
tanh -> -1 and exp -> e^-10, which after the 1/Z normalize is ~5e-15
(reference has exactly 0; far below the accuracy target).

Precision strategy (validated vs fp32 reference: rel L2 ~6e-3):
  - query is rounded once to bf16 (xh); its lo half is dropped.
  - W is kept as bf16 hi+lo ([Wq|Wk] stacked): proj = wh.T xhT + wl.T xhT
    accumulated in fp32 PSUM -> pp = [Q; K] with k on partitions.
  - scores = Qh.T Kh in pure bf16 (64-contraction), no correction terms.
  - xhT is produced by PE transposes (identity matmul) into a bf16 PSUM
    region -- no DRAM-roundtrip DMA transpose.

PSUM layout: one pool of 2 x [P, 2048] fp32 tiles (4 banks each, 8 total).
Per batch, 5 allocations rotate through it: 1 front-end tile (proj
accumulator in fp32 cols 0:512; PE-transpose scratch in a bf16 bitcast
view of cols 1024:1536) + 4 score-pair tiles (2 chunks each). Separate
pool tiles get precise dependency tracking (slices of one big tensor
serialize conservatively); within the front-end tile the ops form a
near-serial chain anyway.

Engine budget per core (target): scalar ~89us (tanh+exp = roofline),
DMA ~78us (24MB output store), PE ~50us, DVE ~60us.
"""

import numpy as np

import concourse.bass as bass
import concourse.bass_isa as bass_isa
import concourse.mybir as mybir
import concourse.tile as tile
from concourse import bacc
from concourse.bass_utils import run_bass_kernel_spmd

B = 48
S = 1024
D = 128
DK = 64
N_CORES = 8
BPC = B // N_CORES
P = 128
NQ = S // P
F32 = mybir.dt.float32
BF16 = mybir.dt.bfloat16
AL = mybir.AluOpType

TANH_CLIP = 10.0
DIAG_NEG = -1.0e9  # pre-tanh diag clamp; tanh(-1e9) = -1 -> exp(-10) ~ 0


def build_bass() -> bass.Bass:
    nc = bacc.Bacc(None, target_bir_lowering=False)

    q_d = nc.dram_tensor("query", [BPC, S, D], F32, kind="ExternalInput")
    wq_d = nc.dram_tensor("W_query", [D, DK], F32, kind="ExternalInput")
    wk_d = nc.dram_tensor("W_key", [D, DK], F32, kind="ExternalInput")
    out_d = nc.dram_tensor("out", [BPC, S, S], F32, kind="ExternalOutput")

    with tile.TileContext(nc) as tc:
        with (
            tc.tile_pool(name="singles", bufs=1) as singles,
            tc.tile_pool(name="qload", bufs=2) as qload,
            tc.tile_pool(name="xhp", bufs=2) as xhp,
            tc.tile_pool(name="xhtp", bufs=2) as xhtp,
            tc.tile_pool(name="hbp", bufs=2) as hbp,
            tc.tile_pool(name="tbuf", bufs=3) as tbuf,
            tc.tile_pool(name="small", bufs=2) as small,
            tc.tile_pool(name="ps", bufs=2, space="PSUM") as ps,
        ):
            # --- one-time setup ---
            # warm the exp/tanh activation table set during the initial DMAs
            warm = singles.tile([P, 1], F32)
            nc.vector.memset(warm, 0.0)
            nc.scalar.activation(
                out=warm, in_=warm, func=mybir.ActivationFunctionType.Tanh
            )

            # bf16 identity for PE transposes
            ident = singles.tile([P, P], BF16)
            nc.vector.memset(ident, 0.0)
            nc.gpsimd.affine_select(
                out=ident,
                in_=ident,
                compare_op=AL.not_equal,
                fill=1.0,
                base=0,
                pattern=[[-1, P]],
                channel_multiplier=1,
            )
            # diag clamp mask: min(scores, dmask) forces diagonal to -1e9
            dmask = singles.tile([P, P], F32)
            nc.vector.memset(dmask, 3.0e38)
            nc.gpsimd.affine_select(
                out=dmask,
                in_=dmask,
                compare_op=AL.not_equal,
                fill=DIAG_NEG,
                base=0,
                pattern=[[-1, P]],
                channel_multiplier=1,
            )

            # W stacked [Wq | Wk] as fp32, then bf16 hi/lo
            w32 = singles.tile([D, 2 * DK], F32)
            nc.sync.dma_start(w32[:, 0:DK], wq_d[:, :])
            nc.sync.dma_start(w32[:, DK:2 * DK], wk_d[:, :])
            wh = singles.tile([D, 2 * DK], BF16)
            nc.vector.tensor_copy(wh, w32)
            wl = singles.tile([D, 2 * DK], BF16)
            nc.vector.tensor_tensor(wl, w32, wh, AL.subtract)

            def load_q(b):
                q_sb = qload.tile([P, NQ, D], F32, tag="q")
                nc.sync.dma_start(
                    q_sb, q_d[b].rearrange("(n p) d -> p n d", p=P)
                )
                return q_sb

            def front_end_a(q_sb):
                """Alloc front-end PSUM tile; cast query to bf16 and
                PE-transpose all 8 chunks into its bf16 region -> xhT."""
                fe = ps.tile([P, 2 * S], F32, tag="ps")
                tp = fe.bitcast(BF16)[:, 2 * S:3 * S]  # fp32 cols 1024:1536
                xh = xhp.tile([P, NQ, D], BF16, tag="xh")
                nc.vector.tensor_copy(xh, q_sb)
                for c in range(NQ):
                    nc.tensor.transpose(
                        tp[:, c * P:(c + 1) * P], xh[:, c, :], ident
                    )
                xhT = xhtp.tile([P, S], BF16, tag="xhT")
                nc.vector.tensor_copy(xhT, tp)
                return fe, xhT

            def front_end_b(fe, xhT):
                """proj pp = [Q;K] (fp32 psum) -> hb bf16; kh = dup of Kh."""
                pp = fe[:, 0:512]
                hb = hbp.tile([P, S], BF16, tag="hb")
                kh = hbp.tile([P, S], BF16, tag="kh")
                for h in range(2):
                    cols = slice(h * 512, (h + 1) * 512)
                    nc.tensor.matmul(pp, wh, xhT[:, cols], start=True, stop=False)
                    nc.tensor.matmul(pp, wl, xhT[:, cols], start=False, stop=True)
                    nc.vector.tensor_copy(hb[:, cols], pp)
                    nc.vector.tensor_copy(kh[0:DK, cols], hb[DK:P, cols])
                return hb, kh

            def score_pair(hb, kh, j):
                """Chunks 2j, 2j+1 into one [P, 2S] PSUM tile; diag-clamp
                both chunks with one strided min."""
                t = ps.tile([P, 2 * S], F32, tag="ps")
                for i in range(2):
                    c = 2 * j + i
                    lhsT = hb[0:DK, c * P:(c + 1) * P]
                    for h in range(2):
                        cols = slice(h * 512, (h + 1) * 512)
                        nc.tensor.matmul(
                            t[:, i * S:][:, cols], lhsT, kh[0:DK, cols],
                            start=True, stop=True,
                        )
                # diag blocks: chunk 2j at cols 2j*P, chunk 2j+1 at
                # S + (2j+1)*P -> stride S+P, 2 blocks of P
                blk0 = t[:, 2 * j * P:(2 * j + 1) * P]
                diag_ap = bass.AP(
                    tensor=blk0.tensor,
                    offset=blk0.offset,
                    ap=[blk0.ap[0], [S + P, 2], [1, P]],
                )
                m0 = dmask[:, 0:P]
                mask_ap = bass.AP(
                    tensor=m0.tensor,
                    offset=m0.offset,
                    ap=[m0.ap[0], [0, 2], [1, P]],
                )
                nc.vector.tensor_tensor(diag_ap, diag_ap, mask_ap, AL.min)
                return t

            def tanh_pair(t_sb, t_ps, j):
                nc.scalar.activation(
                    out=t_sb[:, 2 * j:2 * j + 2],
                    in_=t_ps,
                    func=mybir.ActivationFunctionType.Tanh,
                )

            def exp_batch(t_sb):
                """exp(10*t) in place over the whole batch, row sums -> rs."""
                rs = small.tile([P, 1], F32, tag="rs")
                nc.scalar.activation(
                    out=t_sb,
                    in_=t_sb,
                    func=mybir.ActivationFunctionType.Exp,
                    scale=TANH_CLIP,
                    accum_out=rs,
                )
                zall = small.tile([P, 1], F32, tag="zall")
                nc.gpsimd.partition_all_reduce(
                    zall, rs, channels=P, reduce_op=bass_isa.ReduceOp.add
                )
                rz = small.tile([P, 1], F32, tag="rz")
                nc.vector.reciprocal(rz, zall)
                return rz

            def norm_quarter(t_sb, rz, q):
                """Normalize chunks 2q, 2q+1 (1.1us DVE each, so the diag
                mins are never stuck behind a 4.5us DVE op)."""
                nc.vector.tensor_scalar_mul(
                    t_sb[:, 2 * q:2 * q + 2], t_sb[:, 2 * q:2 * q + 2], rz
                )

            def store_half(b, t_sb, h):
                """big store via SWDGE on the (otherwise idle) GpSimd queue"""
                nc.gpsimd.dma_start(
                    out_d[b].rearrange("(n p) s -> p n s", p=P)[:, 4 * h:4 * h + 4],
                    t_sb[:, 4 * h:4 * h + 4],
                )

            # ---- software-pipelined batch loop --------------------------
            # prologue: batch 0 front-end
            q_sb = load_q(0)
            fe, xhT = front_end_a(q_sb)
            ops = front_end_b(fe, xhT)
            pending = None  # (b, t_sb, rz) awaiting normalize+store

            for b in range(BPC):
                t_sb = tbuf.tile([P, NQ, S], F32, tag="t")
                hb, kh = ops

                if b + 1 < BPC:
                    q_next = load_q(b + 1)

                # pair 0
                tps = score_pair(hb, kh, 0)
                tanh_pair(t_sb, tps, 0)
                # next batch front-end part A (cast + PE transposes)
                if b + 1 < BPC:
                    fe_next, xhT_next = front_end_a(q_next)
                if pending is not None:
                    norm_quarter(pending[1], pending[2], 0)

                # pair 1
                tps = score_pair(hb, kh, 1)
                tanh_pair(t_sb, tps, 1)
                if pending is not None:
                    norm_quarter(pending[1], pending[2], 1)
                    store_half(pending[0], pending[1], 0)
                # next batch front-end part B (proj + hb/kh)
                if b + 1 < BPC:
                    ops = front_end_b(fe_next, xhT_next)

                # pair 2
                tps = score_pair(hb, kh, 2)
                tanh_pair(t_sb, tps, 2)
                if pending is not None:
                    norm_quarter(pending[1], pending[2], 2)

                # pair 3
                tps = score_pair(hb, kh, 3)
                tanh_pair(t_sb, tps, 3)
                if pending is not None:
                    norm_quarter(pending[1], pending[2], 3)
                    store_half(pending[0], pending[1], 1)
                    pending = None

                rz = exp_batch(t_sb)
                pending = (b, t_sb, rz)

            # epilogue
            for q in range(4):
                norm_quarter(pending[1], pending[2], q)
                if q == 1:
                    store_half(pending[0], pending[1], 0)
            store_half(pending[0], pending[1], 1)

    nc.compile()
    return nc


_CACHED_NC = None


def kernel(**inputs: np.ndarray) -> np.ndarray:
    global _CACHED_NC
    query = np.ascontiguousarray(np.asarray(inputs["query"], dtype=np.float32))
    wq = np.ascontiguousarray(np.asarray(inputs["W_query"], dtype=np.float32))
    wk = np.ascontiguousarray(np.asarray(inputs["W_key"], dtype=np.float32))
    assert query.shape == (B, S, D), query.shape

    if _CACHED_NC is None:
        _CACHED_NC = build_bass()
    nc = _CACHED_NC

    in_maps = [
        {
            "query": query[c * BPC:(c + 1) * BPC],
            "W_query": wq,
            "W_key": wk,
        }
        for c in range(N_CORES)
    ]
    res = run_bass_kernel_spmd(nc, in_maps, core_ids=list(range(N_CORES)))
    out = np.concatenate(
        [r["out"].reshape(BPC, S * S) for r in res.results], axis=0
    )
    return out


# revision 8
# speedup vs baseline: 1.4808x; 1.0729x over previous
"""Trainium2 Bass kernel for batched tanh-attention flat-softmax.

Per batch b:
    Q = query[b] @ W_query; K = query[b] @ W_key      # [S, 64]
    s = tanh(Q @ K.T) * 10                            # [S, S]
    s[diag] = -inf
    out[b] = softmax(s.flatten())

Sharding: data-parallel over batch across 8 NeuronCores (6 batches/core),
W_query/W_key replicated; no cross-core communication.

Numerics: tanh(x)*10 is bounded in [-10,10], so softmax needs no max
subtraction: out = exp(10*tanh(s)) / sum(...). The diagonal is clamped to
-1e4 on the tanh output (post-tanh, on SBUF, so the clamp is never on the
PE->tanh critical path), so exp(10*-1e4) underflows to exactly 0, matching
the reference's additive -1e8 mask.

Precision strategy (validated vs fp32 reference: rel L2 ~6e-3):
  - query is rounded once to bf16 (xh); its lo half is dropped.
  - W is kept as bf16 hi+lo ([Wq|Wk] stacked): proj = wh.T xhT + wl.T xhT
    accumulated in fp32 PSUM -> pp = [Q; K] with k on partitions.
  - scores = Qh.T Kh in pure bf16 (64-contraction), no correction terms.
  - xhT is produced by PE transposes (identity matmul) into a bf16 PSUM
    region -- no DRAM-roundtrip DMA transpose.

PSUM layout: score pool of 3 x [P, S] fp32 chunk tiles (2 banks each) +
one [P, 512] fp32 front-end tile (2 banks: proj accumulator in fp32 cols
0:512 bank 0; PE-transpose scratch in a bf16 bitcast view, bank 1) = 8
banks. Separate pool tiles get precise dependency tracking (slices of one
big tensor serialize conservatively). Slot-reuse distance 3 on the score
ring gives ~3.3us of tanh slack per matmul, so the PE never stalls the
scalar engine; the scalar engine streams tanh/exp back-to-back.

Engine budget per core (target): scalar ~97us (tanh+exp = roofline),
DMA ~78us (24MB output store), PE ~50us, DVE ~55us.
"""

import numpy as np

import concourse.bass as bass
import concourse.bass_isa as bass_isa
import concourse.mybir as mybir
import concourse.tile as tile
from concourse import bacc
from concourse.bass_utils import run_bass_kernel_spmd

B = 48
S = 1024
D = 128
DK = 64
N_CORES = 8
BPC = B // N_CORES
P = 128
NQ = S // P
F32 = mybir.dt.float32
BF16 = mybir.dt.bfloat16
AL = mybir.AluOpType

TANH_CLIP = 10.0
DIAG_NEG = -1.0e4  # post-tanh diag clamp; exp(10 * -1e4) == 0 exactly


def build_bass() -> bass.Bass:
    nc = bacc.Bacc(None, target_bir_lowering=False)

    q_d = nc.dram_tensor("query", [BPC, S, D], F32, kind="ExternalInput")
    wq_d = nc.dram_tensor("W_query", [D, DK], F32, kind="ExternalInput")
    wk_d = nc.dram_tensor("W_key", [D, DK], F32, kind="ExternalInput")
    out_d = nc.dram_tensor("out", [BPC, S, S], F32, kind="ExternalOutput")

    with tile.TileContext(nc) as tc:
        with (
            tc.tile_pool(name="singles", bufs=1) as singles,
            tc.tile_pool(name="qload", bufs=3) as qload,
            tc.tile_pool(name="xhp", bufs=2) as xhp,
            tc.tile_pool(name="xhtp", bufs=2) as xhtp,
            tc.tile_pool(name="hbp", bufs=2) as hbp,
            tc.tile_pool(name="tbuf", bufs=3) as tbuf,
            tc.tile_pool(name="small", bufs=2) as small,
            tc.tile_pool(name="ps", bufs=3, space="PSUM") as ps,
            tc.tile_pool(name="psfe", bufs=1, space="PSUM") as psfe,
        ):
            # --- one-time setup ---
            # warm the exp/tanh activation table set during the initial DMAs
            warm = singles.tile([P, 1], F32)
            nc.vector.memset(warm, 0.0)
            nc.scalar.activation(
                out=warm, in_=warm, func=mybir.ActivationFunctionType.Tanh
            )

            def load_q(b):
                q_sb = qload.tile([P, NQ, D], F32, tag="q")
                nc.sync.dma_start(
                    q_sb, q_d[b].rearrange("(n p) d -> p n d", p=P)
                )
                return q_sb

            # query loads go out first: the front-end consumes them earliest
            q_loaded = [load_q(0), load_q(1)]

            # bf16 identity for PE transposes
            ident = singles.tile([P, P], BF16)
            nc.vector.memset(ident, 0.0)
            nc.gpsimd.affine_select(
                out=ident,
                in_=ident,
                compare_op=AL.not_equal,
                fill=1.0,
                base=0,
                pattern=[[-1, P]],
                channel_multiplier=1,
            )
            # diag clamp mask: min(tanh_out, dmask) forces diagonal to -1e4
            dmask = singles.tile([P, P], F32)
            nc.vector.memset(dmask, 3.0e38)
            nc.gpsimd.affine_select(
                out=dmask,
                in_=dmask,
                compare_op=AL.not_equal,
                fill=DIAG_NEG,
                base=0,
                pattern=[[-1, P]],
                channel_multiplier=1,
            )

            # W stacked [Wq | Wk] as fp32, then bf16 hi/lo
            w32 = singles.tile([D, 2 * DK], F32)
            nc.sync.dma_start(w32[:, 0:DK], wq_d[:, :])
            nc.sync.dma_start(w32[:, DK:2 * DK], wk_d[:, :])
            wh = singles.tile([D, 2 * DK], BF16)
            nc.vector.tensor_copy(wh, w32)
            wl = singles.tile([D, 2 * DK], BF16)
            nc.vector.tensor_tensor(wl, w32, wh, AL.subtract)

            # persistent 2-bank front-end PSUM tile: proj accumulator in
            # fp32 cols 0:512 (bank 0), PE-transpose scratch as bf16 view
            # of fp32 cols 512:1024 (bank 1)
            fe = psfe.tile([P, S], F32)
            pp = fe[:, 0:512]
            tp = fe.bitcast(BF16)[:, S:2 * S]

            def front_end_a(q_sb):
                """Cast query to bf16, PE-transpose all 8 chunks -> xhT."""
                xh = xhp.tile([P, NQ, D], BF16, tag="xh")
                nc.vector.tensor_copy(xh, q_sb)
                for c in range(NQ):
                    nc.tensor.transpose(
                        tp[:, c * P:(c + 1) * P], xh[:, c, :], ident
                    )
                xhT = xhtp.tile([P, S], BF16, tag="xhT")
                nc.vector.tensor_copy(xhT, tp)
                return xhT

            def front_end_b(xhT):
                """proj pp = [Q;K] (fp32 psum) -> hb bf16; kh = dup of Kh."""
                hb = hbp.tile([P, S], BF16, tag="hb")
                kh = hbp.tile([P, S], BF16, tag="kh")
                for h in range(2):
                    cols = slice(h * 512, (h + 1) * 512)
                    nc.tensor.matmul(pp, wh, xhT[:, cols], start=True, stop=False)
                    nc.tensor.matmul(pp, wl, xhT[:, cols], start=False, stop=True)
                    nc.vector.tensor_copy(hb[:, cols], pp)
                    nc.vector.tensor_copy(kh[0:DK, cols], hb[DK:P, cols])
                return hb, kh

            def score_tanh_chunk(hb, kh, t_sb, c):
                """Chunk c: 2 matmuls into a [P, S] PSUM ring tile, then
                tanh straight to t_sb. Nothing but the PE feeds the tanh."""
                t = ps.tile([P, S], F32, tag="sc")
                lhsT = hb[0:DK, c * P:(c + 1) * P]
                for h in range(2):
                    cols = slice(h * 512, (h + 1) * 512)
                    nc.tensor.matmul(
                        t[:, cols], lhsT, kh[0:DK, cols], start=True, stop=True
                    )
                nc.scalar.activation(
                    out=t_sb[:, c],
                    in_=t,
                    func=mybir.ActivationFunctionType.Tanh,
                )

            def diag_clamp_pair(t_sb, j):
                """Clamp diag blocks of chunks 2j, 2j+1 on the tanh output
                (SBUF) with one strided min; off the tanh critical path,
                only exp depends on it."""
                blk0 = t_sb[:, 2 * j, 2 * j * P:(2 * j + 1) * P]
                diag_ap = bass.AP(
                    tensor=blk0.tensor,
                    offset=blk0.offset,
                    ap=[blk0.ap[0], [S + P, 2], [1, P]],
                )
                m0 = dmask[:, 0:P]
                mask_ap = bass.AP(
                    tensor=m0.tensor,
                    offset=m0.offset,
                    ap=[m0.ap[0], [0, 2], [1, P]],
                )
                nc.vector.tensor_tensor(diag_ap, diag_ap, mask_ap, AL.min)

            def exp_batch(t_sb):
                """exp(10*t) in place over the whole batch, row sums -> rs."""
                rs = small.tile([P, 1], F32, tag="rs")
                nc.scalar.activation(
                    out=t_sb,
                    in_=t_sb,
                    func=mybir.ActivationFunctionType.Exp,
                    scale=TANH_CLIP,
                    accum_out=rs,
                )
                zall = small.tile([P, 1], F32, tag="zall")
                nc.gpsimd.partition_all_reduce(
                    zall, rs, channels=P, reduce_op=bass_isa.ReduceOp.add
                )
                rz = small.tile([P, 1], F32, tag="rz")
                nc.vector.reciprocal(rz, zall)
                return rz

            def norm_quarter(t_sb, rz, q):
                """Normalize chunks 2q, 2q+1 (1.1us DVE each, so the diag
                mins are never stuck behind a 4.5us DVE op)."""
                nc.vector.tensor_scalar_mul(
                    t_sb[:, 2 * q:2 * q + 2], t_sb[:, 2 * q:2 * q + 2], rz
                )

            def store_half(b, t_sb, h):
                """big store via SWDGE on the (otherwise idle) GpSimd queue"""
                nc.gpsimd.dma_start(
                    out_d[b].rearrange("(n p) s -> p n s", p=P)[:, 4 * h:4 * h + 4],
                    t_sb[:, 4 * h:4 * h + 4],
                )

            # ---- software-pipelined batch loop --------------------------
            # prologue: batch 0 front-end (q0/q1 loads already in flight)
            xhT = front_end_a(q_loaded[0])
            ops = front_end_b(xhT)
            pending = None  # (b, t_sb, rz) awaiting normalize+store

            for b in range(BPC):
                t_sb = tbuf.tile([P, NQ, S], F32, tag="t")
                hb, kh = ops

                if b + 2 < BPC:
                    q_loaded.append(load_q(b + 2))

                score_tanh_chunk(hb, kh, t_sb, 0)
                score_tanh_chunk(hb, kh, t_sb, 1)
                diag_clamp_pair(t_sb, 0)
                # next batch front-end part A (cast + PE transposes)
                if b + 1 < BPC:
                    xhT_next = front_end_a(q_loaded[b + 1])
                if pending is not None:
                    norm_quarter(pending[1], pending[2], 0)

                score_tanh_chunk(hb, kh, t_sb, 2)
                score_tanh_chunk(hb, kh, t_sb, 3)
                diag_clamp_pair(t_sb, 1)
                if pending is not None:
                    norm_quarter(pending[1], pending[2], 1)
                    store_half(pending[0], pending[1], 0)
                # next batch front-end part B (proj + hb/kh)
                if b + 1 < BPC:
                    ops = front_end_b(xhT_next)

                score_tanh_chunk(hb, kh, t_sb, 4)
                score_tanh_chunk(hb, kh, t_sb, 5)
                diag_clamp_pair(t_sb, 2)
                if pending is not None:
                    norm_quarter(pending[1], pending[2], 2)

                score_tanh_chunk(hb, kh, t_sb, 6)
                score_tanh_chunk(hb, kh, t_sb, 7)
                diag_clamp_pair(t_sb, 3)
                if pending is not None:
                    norm_quarter(pending[1], pending[2], 3)
                    store_half(pending[0], pending[1], 1)
                    pending = None

                rz = exp_batch(t_sb)
                pending = (b, t_sb, rz)

            # epilogue
            for q in range(4):
                norm_quarter(pending[1], pending[2], q)
                if q == 1:
                    store_half(pending[0], pending[1], 0)
            store_half(pending[0], pending[1], 1)

    nc.compile()
    return nc


_CACHED_NC = None


def kernel(**inputs: np.ndarray) -> np.ndarray:
    global _CACHED_NC
    query = np.ascontiguousarray(np.asarray(inputs["query"], dtype=np.float32))
    wq = np.ascontiguousarray(np.asarray(inputs["W_query"], dtype=np.float32))
    wk = np.ascontiguousarray(np.asarray(inputs["W_key"], dtype=np.float32))
    assert query.shape == (B, S, D), query.shape

    if _CACHED_NC is None:
        _CACHED_NC = build_bass()
    nc = _CACHED_NC

    in_maps = [
        {
            "query": query[c * BPC:(c + 1) * BPC],
            "W_query": wq,
            "W_key": wk,
        }
        for c in range(N_CORES)
    ]
    res = run_bass_kernel_spmd(nc, in_maps, core_ids=list(range(N_CORES)))
    out = np.concatenate(
        [r["out"].reshape(BPC, S * S) for r in res.results], axis=0
    )
    return out
